# revision 38
# baseline (speedup 1.0000x reference)
"""Trainium2 Bass kernel for nn_BaseEBM (EBM inner gradient-descent loop).

Computation (per sample, matching the reference):
    y = y_mean
    repeat 20x:  y <- y - 0.1 * dE/dy
    E(x, y) = W3 @ relu(W2 @ relu(W1 @ relu(W0 @ [x, y] + b0) + b1) + b2) + b3

Distribution: pure data parallel over 8 NeuronCores (131072 samples each).

Device layout: feature-major [128, 512] tiles. Four independent sample
groups of 32 features are packed INTERLEAVED across the 128 partitions
(partition = 4*feature + group); 512 samples per group per tile -> 2048
samples/tile, 64 tiles/core. All matmuls use kron(A, I4) weights so one
instruction processes 4 groups at full PE rate (fp32r, 1 cycle/column).

Key algebraic restructurings:
  * The energy head (layer 3 forward) is never computed; W3 is folded into
    the first backward matmul: g1p = blkdiag(W2 * W3^T) @ m2.
  * x never changes across steps, so z0 = W0 @ [x, y] is kept resident in
    PSUM for all 20 steps and updated by accumulating matmuls:
        dz0 = -lr * w0y (w0y . g0) = blkdiag(P) @ g0,  P = -lr w0y w0y^T.
  * y is never materialized during the loop.  Since dz0 = w0y * dy, the
    final y is recovered from the PSUM residue:
        y = (z0_fin[f*] - W0[f*,0] x) / w0y[f*]   (y_mean is inside z0).
    This frees a PSUM bank per chain, allowing FOUR independent
    tile-chains in the 8 banks - needed because one chain's per-step
    dependency path (~4.5 us) is ~3x its per-engine work (~1.6 us).
  * Layer-0 features are host-permuted so f* is feature 0; with the
    interleaved layout the f*-rows are partitions 0..3, so the final y is
    one DVE op reading the z0 bank directly (no 128-row copy / gathers).
  * Masked backprop uses the fused DVE op (h > 0) * g in one instruction
    (scalar_tensor_tensor with is_gt + mult), so relu masks are never
    materialized for layers 0/1.
  * The layer-2 mask is ONE op either way: ACT sigmoid(2^100 * z2) (exact
    0/1 - sigmoid saturates) or DVE is_gt from PSUM; a per-chain-offset
    60/40 route split balances ACT vs DVE load at every instant.
  * The transient-PSUM pool is bufs=1 per chain: the slot-reuse deps
    exactly coincide with the data deps (z2 can only start after h1,
    which is when z1's bank frees), so one bank per chain costs nothing.

Per step per tile: 6 matmuls (V in split-bf16: hi+lo, needed for the
2e-2 error budget), 2 ACT relus, 1 one-op layer-2 mask, 2 fused DVE
mask-multiplies.  ACT and DVE are the bottleneck engines (~87% busy in
the cost-model timeline); PE ~67%.
"""

import numpy as np

import concourse.bass as bass
import concourse.mybir as mybir
import concourse.tile as tile
from concourse import bacc
from concourse.bass_utils import run_bass_kernel_spmd

F32 = mybir.dt.float32
F32R = mybir.dt.float32r
BF16 = mybir.dt.bfloat16
ALU = mybir.AluOpType
AF = mybir.ActivationFunctionType

B = 1048576
NCORES = 8
BC = B // NCORES           # 131072 samples per core
G = 4                      # sample groups packed across partitions
TILE_N = 512               # samples per group per tile (PSUM bank limit)
SPT = G * TILE_N           # 2048 samples per tile
NT_FULL = BC // SPT        # 64 tiles per core
STEPS = 20
LR = 0.1
W = 32
NCHAINS = 4
SHARED_TMP = False
TMP_BUFS = 3
DYN = True      # hardware For_i loop over tile-quads
FWD_FP32 = False  # true-fp32 forward matmuls (4 cyc/row) -> ~40x less error
MASK_SCALE = float(2.0 ** 100)  # sigmoid(MASK_SCALE*z) == (z > 0) exactly
# m2-route table [step % 5][chain % 4]: True = ACT sigmoid, False = DVE
# is_gt.  Each chain is 3/5 ACT; the per-chain offset staggers routes so
# the engines see a mixed load at every instant (all chains on the same
# route bunches one engine and idles the other for ~650ns per step).
import os as _os
_RV = _os.environ.get("K_ROUTE", "coset")
if _RV == "coset":
    M2_ROUTE = [[(s + 2 * c) % 5 < 3 for c in range(4)] for s in range(5)]
else:
    M2_ROUTE = [
        [True, True, True, False],
        [True, False, False, True],
        [False, True, True, True],
        [True, False, True, False],
        [False, True, False, True],
    ]
PAIR_EMIT = _os.environ.get("K_PAIR", "1") == "1"
IO_BUFS = int(_os.environ.get("K_IOBUFS", "2"))
PRIME = int(_os.environ.get("K_PRIME", "5"))
VMM = _os.environ.get("K_VMM", "split2")  # split2 | bf16 | f32r
_SHTMP = int(_os.environ.get("K_SHTMP", "0"))
if _SHTMP:
    SHARED_TMP = True
    TMP_BUFS = _SHTMP



def _emit_tile_chain(nc, t, c, dram, wt, sb, ptmp, pz0, io, fstar, inv, cfac):
    """Generator emitting one packed tile's program; yields between steps
    so NCHAINS chains interleave in emission (and thus in the static
    per-engine schedules)."""
    _dyn = not isinstance(t, int)
    src = dram["inp0"][bass.ds(t, 1)][0, c] if _dyn else dram["inp0"][t][c]
    dst = dram["yout"][bass.ds(t, 1)][0, c] if _dyn else dram["yout"][t][c]
    # Host-side feature permutation puts f* at feature 0, so with the
    # interleaved layout (partition = 4*feature + group) the extraction
    # rows are partitions 0..3 (32-aligned base, required by the engines).
    assert fstar == 0
    inp = io.tile([2 * G, TILE_N], F32 if FWD_FP32 else F32R,
                  tag=f"inp{c}", name=f"inp_{c}")
    nc.sync.dma_start(out=inp[:], in_=src)
    xt = io.tile([G, TILE_N], F32, tag=f"xt{c}", name=f"xt_{c}")
    nc.sync.dma_start(out=xt[:], in_=src[0::2, :].bitcast(F32))

    z0 = pz0.tile([128, TILE_N], F32, tag="z0", name=f"z0_{c}")
    # z0 = blkdiag(W0) @ [x; y_mean]   (no bias; ACT adds b0 every step)
    nc.tensor.matmul(
        z0[:], wt["L0"][:], inp[:],
        start=True, stop=False, skip_group_check=True,
    )
    yield

    for s in range(STEPS):
        HDT = F32 if FWD_FP32 else F32R
        h0 = sb.tile([128, TILE_N], HDT, tag="h0", name=f"h0_{c}")
        nc.scalar.activation(h0[:], z0[:], AF.Relu, bias=wt["b0"][:])
        yield
        z1 = ptmp.tile([128, TILE_N], F32, tag="tmp", name=f"z1_{c}")
        nc.tensor.matmul(
            z1[:], wt["Lz1"][:], h0[:],
            start=True, stop=True, skip_group_check=True,
        )
        yield
        h1 = sb.tile([128, TILE_N], HDT, tag="h1", name=f"h1_{c}")
        nc.scalar.activation(h1[:], z1[:], AF.Relu, bias=wt["b1"][:])
        yield
        z2 = ptmp.tile([128, TILE_N], F32, tag="tmp", name=f"z2_{c}")
        nc.tensor.matmul(
            z2[:], wt["Lz2"][:], h1[:],
            start=True, stop=True, skip_group_check=True,
        )
        yield
        m2 = sb.tile([128, TILE_N], BF16, tag="m2", name=f"m2_{c}")
        # GpSimd tensor_scalar measured ~8us/op on HW - never use it.
        # The 0/1 mask is exact in bf16.  m2 = (z2 + b2 > 0) is computed in
        # ONE op either on ACT as sigmoid(2^100 * z2 + 2^100*b2) (sigmoid
        # saturates to exact 0.0/1.0 for |arg| > ~90, i.e. |z2+b2| >
        # ~7e-29) or on DVE as is_gt straight from PSUM; the split
        # balances ACT vs DVE load (~60/40).  The 5x4 table staggers the
        # routes across chains: every chain is 3/5 ACT, and the per-step
        # ACT-lane counts are [3,2,3,2,2] - putting all chains on the same
        # route bunches one engine and idles the other for ~650ns per step.
        if M2_ROUTE[s % 5][c % 4]:
            nc.scalar.activation(m2[:], z2[:], AF.Sigmoid,
                                 bias=wt["sb2"][:], scale=MASK_SCALE)
        else:
            nc.vector.tensor_scalar(m2[:], z2[:], wt["nb2"][:], None,
                                    ALU.is_gt)
        yield
        # g1p = blkdiag(W2 * W3^T) @ m2 in split-bf16 (hi + lo residual,
        # ~16-bit effective weights - better than fp32r).
        g1p = ptmp.tile([128, TILE_N], F32, tag="tmp", name=f"g1p_{c}")
        if VMM == "split2":
            nc.tensor.matmul(
                g1p[:], wt["Lg1h"][:], m2[:],
                start=True, stop=False, skip_group_check=True,
            )
            nc.tensor.matmul(
                g1p[:], wt["Lg1l"][:], m2[:],
                start=False, stop=True, skip_group_check=True,
            )
        else:
            nc.tensor.matmul(
                g1p[:], wt["Lg1h" if VMM == "bf16" else "Lg1f"][:], m2[:],
                start=True, stop=True, skip_group_check=True,
            )
        yield
        g1 = sb.tile([128, TILE_N], F32R, tag="g1", name=f"g1_{c}")
        nc.vector.scalar_tensor_tensor(
            g1[:], h1[:], 0.0, g1p[:], op0=ALU.is_gt, op1=ALU.mult
        )
        yield
        g0p = ptmp.tile([128, TILE_N], F32, tag="tmp", name=f"g0p_{c}")
        nc.tensor.matmul(
            g0p[:], wt["Lg0"][:], g1[:],
            start=True, stop=True, skip_group_check=True,
        )
        yield
        g0 = sb.tile([128, TILE_N], F32R, tag="g0", name=f"g0_{c}")
        nc.vector.scalar_tensor_tensor(
            g0[:], h0[:], 0.0, g0p[:], op0=ALU.is_gt, op1=ALU.mult
        )
        yield
        # z0 += blkdiag(P) @ g0  == w0y (x) dy for this step
        nc.tensor.matmul(
            z0[:], wt["LP"][:], g0[:],
            start=False, stop=(s == STEPS - 1), skip_group_check=True,
        )
        yield

    # y = (z0_fin[f*] - W0[f*,0]*x) * inv + y_mean   (inv = 1/W0[f*,1]).
    # The interleaved layout + f*-first feature permutation make the four
    # f*-rows partitions 0..3, so the final y comes straight from the z0
    # PSUM bank - no 128-row copy, no row-gather DMAs.
    t1 = io.tile([G, TILE_N], F32, tag=f"t1{c}", name=f"t1_{c}")
    # t1 = x * (W0[f*,0]*inv);  y_mean cancels: z0_init[f*] already
    # includes W0[f*,1]*y_mean, so y = z0[f*]*inv - x*cfac exactly.
    nc.scalar.activation(t1[:], xt[:], AF.Copy, scale=cfac)
    yield
    yo = io.tile([G, TILE_N], F32, tag=f"yo{c}", name=f"yo_{c}")
    # yo = z0[f*-rows] * inv - t1   (also releases the z0 bank)
    nc.vector.scalar_tensor_tensor(yo[:], z0[0:G, :], inv, t1[:],
                                   op0=ALU.mult, op1=ALU.subtract)
    yield
    nc.sync.dma_start(out=dst, in_=yo[:])
    yield


def build(nt=NT_FULL, fstar=0, inv=1.0, cfac=1.0, reps=1, dyn=None):
    """Build + compile the per-core Bass program for nt packed tiles."""
    nc = bacc.Bacc("TRN2", target_bir_lowering=False, debug=False,
                   num_devices=NCORES)

    ntq = nt // NCHAINS
    dram = {
        "inp0": nc.dram_tensor("inp0", [ntq, NCHAINS, 2 * G, TILE_N],
                               F32 if FWD_FP32 else F32R,
                               kind="ExternalInput").ap(),
        "yout": nc.dram_tensor("yout", [ntq, NCHAINS, G, TILE_N], F32,
                               kind="ExternalOutput").ap(),
    }
    wspec = {
        "L0": [2 * G, 128],
        "Lz1": [128, 128], "Lz2": [128, 128],
        "Lg1h": [128, 128], "Lg1l": [128, 128], "Lg1f": [128, 128],
        "Lg0": [128, 128],
        "LP": [128, 128],
        "b0": [128, 1], "b1": [128, 1], "b2": [128, 1], "nb2": [128, 1],
        "sb2": [128, 1],
        "ym": [G, 1],
    }
    fwd = F32 if FWD_FP32 else F32R
    wdtype = {k: (F32 if k in ("b0", "b1", "b2", "nb2", "sb2", "ym") else
                  (BF16 if k in ("Lg1h", "Lg1l") else
                   (fwd if k in ("Lz1", "Lz2", "L0") else F32R)))
              for k in wspec}
    wdram = {k: nc.dram_tensor(f"w_{k}", sh, wdtype[k],
                               kind="ExternalInput").ap()
             for k, sh in wspec.items()}

    with tile.TileContext(nc) as tc:
        import contextlib
        with contextlib.ExitStack() as ctx:
            wp = ctx.enter_context(tc.tile_pool(name="wp", bufs=1))
            io = ctx.enter_context(tc.tile_pool(name="io", bufs=IO_BUFS))
            sbs = [ctx.enter_context(tc.tile_pool(name=f"sb{c}", bufs=2))
                   for c in range(NCHAINS)]
            if SHARED_TMP:
                pt = ctx.enter_context(
                    tc.tile_pool(name="pt", bufs=TMP_BUFS, space="PSUM"))
                ptmps = [pt] * NCHAINS
            else:
                ptmps = [ctx.enter_context(
                    tc.tile_pool(name=f"pt{c}", bufs=1, space="PSUM"))
                    for c in range(NCHAINS)]
            pz0s = [ctx.enter_context(
                tc.tile_pool(name=f"pz{c}", bufs=1, space="PSUM"))
                for c in range(NCHAINS)]

            wt = {}
            for k, sh in wspec.items():
                wt[k] = wp.tile(sh, wdtype[k], tag=f"w_{k}", name=f"wt_{k}")
                nc.sync.dma_start(out=wt[k][:], in_=wdram[k][:])

            assert nt % NCHAINS == 0

            def emit_pair(t0, t1):
                # Each lane runs its tile in quad t0 then its tile in quad
                # t1 as ONE continuous generator, so a lane's tile boundary
                # (extraction + restart) is surrounded in every engine's
                # program order by the OTHER lanes' steady-state step ops -
                # bunching all 4 boundaries together starves ACT/DVE for
                # ~650ns per boundary.
                def lane(c):
                    yield from _emit_tile_chain(nc, t0, c, dram, wt,
                                               sbs[c], ptmps[c], pz0s[c],
                                               io, fstar, inv, cfac)
                    yield from _emit_tile_chain(nc, t1, c, dram, wt,
                                               sbs[c], ptmps[c], pz0s[c],
                                               io, fstar, inv, cfac)

                def quad_lane(c, t):
                    yield from _emit_tile_chain(nc, t, c, dram, wt,
                                               sbs[c], ptmps[c], pz0s[c],
                                               io, fstar, inv, cfac)
                if PAIR_EMIT:
                    chains = [lane(c) for c in range(NCHAINS)]
                else:
                    chains = None
                # phase-offset the chains by ~1/NCHAINS of a step so no
                # engine sees two dependent ops of one chain back-to-back
                prime = PRIME
                if chains is None:
                    for t in (t0, t1):
                        chs = [quad_lane(c, t) for c in range(NCHAINS)]
                        for c, ch in enumerate(chs):
                            for _ in range(c * prime):
                                next(ch)
                        alive = list(chs)
                        while alive:
                            for ch in list(alive):
                                try:
                                    next(ch)
                                except StopIteration:
                                    alive.remove(ch)
                    return
                for c, ch in enumerate(chains):
                    for _ in range(c * prime):
                        next(ch)
                alive = list(chains)
                while alive:
                    for ch in list(alive):
                        try:
                            next(ch)
                        except StopIteration:
                            alive.remove(ch)

            use_dyn = DYN if dyn is None else dyn
            if use_dyn:
                def body():
                    assert ntq % 2 == 0
                    with tc.For_i(0, ntq // 2, 1,
                                  hint_engines=(mybir.EngineType.PE,)) as iv:
                        iv2 = iv * 2
                        emit_pair(iv2, iv2 + 1)
                if reps > 1:
                    with tc.For_i(0, reps, 1):
                        body()
                else:
                    body()
            else:
                for tq in range(0, ntq, 2):
                    emit_pair(tq, tq + 1)

    nc.compile()
    return nc


def make_weight_arrays(W0, b0, W1, b1, W2, b2, W3, b3, y_mean):
    """Host-side constant construction (all small)."""
    import ml_dtypes
    LR0 = MASK_SCALE
    # Layer-0 features are permuted so f* = argmax |W0[:,1]| is feature 0
    # (the extraction rows land on 32-aligned partitions 0..3), then laid
    # out interleaved: partition = 4*feature + group, so blkdiag weights
    # are kron(A, I_G) and per-feature biases repeat 4x.
    W0 = np.asarray(W0, np.float32)
    f_raw = int(np.argmax(np.abs(W0[:, 1])))
    perm = [f_raw] + [j for j in range(W) if j != f_raw]
    W0 = W0[perm]
    b0 = np.asarray(b0, np.float32)[perm]
    W1 = np.asarray(W1, np.float32)[:, perm]
    eye = np.eye(G, dtype=np.float32)
    blk = lambda A: np.kron(A.astype(np.float32), eye)
    rep = lambda v: np.repeat(v.astype(np.float32), G)[:, None]
    w0y = W0[:, 1].astype(np.float32)
    P = (-LR) * np.outer(w0y, w0y)
    ym = np.float32(np.asarray(y_mean).reshape(-1)[0])
    V = blk(W2 * W3[0][:, None])
    Vh = V.astype(ml_dtypes.bfloat16)
    Vl = (V - Vh.astype(np.float32)).astype(ml_dtypes.bfloat16)
    L0 = np.zeros((2 * G, 128), np.float32)
    for g in range(G):
        L0[2 * g, g::G] = W0[:, 0]          # x_g feeds z0[4j+g]
        L0[2 * g + 1, g::G] = W0[:, 1]      # y_g feeds z0[4j+g]
    out = {
        "w_L0": L0,                              # [8, 128]
        "w_Lz1": blk(W1.T),                      # [128, 128]
        "w_Lz2": blk(W2.T),                      # [128, 128]
        "w_Lg1h": None, "w_Lg1l": None,          # filled below (bf16 pair)
        "w_Lg1f": V,                             # fp32r single-MM variant
        "w_Lg0": blk(W1),                        # [128, 128]
        "w_LP": blk(P),                          # [128, 128]
        "w_b0": rep(b0),
        "w_b1": rep(b1),
        "w_b2": rep(b2),
        "w_nb2": rep(-b2),
        "w_sb2": np.clip(np.repeat(b2.astype(np.float64), G)[:, None] * LR0,
                         -3e38, 3e38).astype(np.float32),
        "w_ym": np.full((G, 1), ym, np.float32),
    }
    out["w_Lg1h"] = Vh
    out["w_Lg1l"] = Vl
    return out


def extraction_consts(W0):
    """inv/cfac for the f* feature; fstar returned as 0 because
    make_weight_arrays permutes f* to feature position 0."""
    W0 = np.asarray(W0, np.float32)
    f_raw = int(np.argmax(np.abs(W0[:, 1])))
    inv = float(1.0 / W0[f_raw, 1])
    cfac = float(W0[f_raw, 0] * inv)
    return 0, inv, cfac


def make_core_inputs(x, y_mean, nt=NT_FULL):
    """Per-core input tiles: [nt, 8, 512] with x on even rows, y_mean on
    odd rows.  Returns a list of NCORES arrays."""
    xs = np.ascontiguousarray(
        np.asarray(x, np.float32).reshape(NCORES, nt, G, TILE_N))
    ym = np.float32(np.asarray(y_mean).reshape(-1)[0])
    maps = []
    for c in range(NCORES):
        inp0 = np.empty((nt, 2 * G, TILE_N), dtype=np.float32)
        inp0[:, 0::2, :] = xs[c]
        inp0[:, 1::2, :] = ym
        maps.append(inp0.reshape(nt // NCHAINS, NCHAINS, 2 * G, TILE_N))
    return maps


_NC_CACHE = {}


def get_nc(nt, fstar, inv, cfac):
    key = (nt, fstar, round(inv, 9), round(cfac, 9))
    if key not in _NC_CACHE:
        _NC_CACHE[key] = build(nt, fstar, inv, cfac)
    return _NC_CACHE[key]


def kernel(x, W0, b0, W1, b1, W2, b2, W3, b3, y_mean):
    x = np.asarray(x, dtype=np.float32)
    fstar, inv, cfac = extraction_consts(W0)
    nc = get_nc(NT_FULL, fstar, inv, cfac)

    warr = make_weight_arrays(
        np.asarray(W0), np.asarray(b0), np.asarray(W1), np.asarray(b1),
        np.asarray(W2), np.asarray(b2), np.asarray(W3), np.asarray(b3),
        np.asarray(y_mean))
    inp0s = make_core_inputs(x, np.asarray(y_mean), NT_FULL)
    in_maps = [{"inp0": inp0s[c], **warr} for c in range(NCORES)]

    res = run_bass_kernel_spmd(nc, in_maps, list(range(NCORES)))
    youts = [res.results[c]["yout"].reshape(BC) for c in range(NCORES)]
    return np.concatenate(youts).reshape(B, 1).astype(np.float32)



# revision 46
# speedup vs baseline: 1.0155x; 1.0155x over previous
"""Trainium2 Bass kernel for nn_BaseEBM (EBM inner gradient-descent loop).

Computation (per sample, matching the reference):
    y = y_mean
    repeat 20x:  y <- y - 0.1 * dE/dy
    E(x, y) = W3 @ relu(W2 @ relu(W1 @ relu(W0 @ [x, y] + b0) + b1) + b2) + b3

Distribution: pure data parallel over 8 NeuronCores (131072 samples each).

Device layout: feature-major [128, 512] tiles. Four independent sample
groups of 32 features are packed INTERLEAVED across the 128 partitions
(partition = 4*feature + group); 512 samples per group per tile -> 2048
samples/tile, 64 tiles/core. All matmuls use kron(A, I4) weights so one
instruction processes 4 groups at full PE rate (fp32r, 1 cycle/column).

Key algebraic restructurings:
  * The energy head (layer 3 forward) is never computed; W3 is folded into
    the first backward matmul: g1p = blkdiag(W2 * W3^T) @ m2.
  * x never changes across steps, so z0 = W0 @ [x, y] is kept resident in
    PSUM for all 20 steps and updated by accumulating matmuls:
        dz0 = -lr * w0y (w0y . g0) = blkdiag(P) @ g0,  P = -lr w0y w0y^T.
  * y is never materialized during the loop.  Since dz0 = w0y * dy, the
    final y is recovered from the PSUM residue:
        y = (z0_fin[f*] - W0[f*,0] x) / w0y[f*]   (y_mean is inside z0).
    This frees a PSUM bank per chain, allowing FOUR independent
    tile-chains in the 8 banks - needed because one chain's per-step
    dependency path (~4.5 us) is ~3x its per-engine work (~1.6 us).
  * Layer-0 features are host-permuted so f* is feature 0; with the
    interleaved layout the f*-rows are partitions 0..3, so the final y is
    one DVE op reading the z0 bank directly (no 128-row copy / gathers).
  * Masked backprop uses the fused DVE op (h > 0) * g in one instruction
    (scalar_tensor_tensor with is_gt + mult), so relu masks are never
    materialized for layers 0/1.
  * The layer-2 mask is ONE op either way: ACT sigmoid(2^100 * z2) (exact
    0/1 - sigmoid saturates) or DVE is_gt from PSUM; a per-chain-offset
    60/40 route split balances ACT vs DVE load at every instant.
  * The transient-PSUM pool is bufs=1 per chain: the slot-reuse deps
    exactly coincide with the data deps (z2 can only start after h1,
    which is when z1's bank frees), so one bank per chain costs nothing.

Per step per tile: 6 matmuls (V in split-bf16: hi+lo, needed for the
2e-2 error budget), 2 ACT relus, 1 one-op layer-2 mask, 2 fused DVE
mask-multiplies.  ACT and DVE are the bottleneck engines (~87% busy in
the cost-model timeline); PE ~67%.
"""

import numpy as np

import concourse.bass as bass
import concourse.mybir as mybir
import concourse.tile as tile
from concourse import bacc
from concourse.bass_utils import run_bass_kernel_spmd

F32 = mybir.dt.float32
F32R = mybir.dt.float32r
BF16 = mybir.dt.bfloat16
ALU = mybir.AluOpType
AF = mybir.ActivationFunctionType

B = 1048576
NCORES = 8
BC = B // NCORES           # 131072 samples per core
G = 4                      # sample groups packed across partitions
TILE_N = 512               # samples per group per tile (PSUM bank limit)
SPT = G * TILE_N           # 2048 samples per tile
NT_FULL = BC // SPT        # 64 tiles per core
STEPS = 20
LR = 0.1
W = 32
NCHAINS = 4
SHARED_TMP = False
TMP_BUFS = 3
DYN = True      # hardware For_i loop over tile-quads
FWD_FP32 = False  # true-fp32 forward matmuls (4 cyc/row) -> ~40x less error
MASK_SCALE = float(2.0 ** 100)  # sigmoid(MASK_SCALE*z) == (z > 0) exactly
# m2-route table [step % 5][chain % 4]: True = ACT sigmoid, False = DVE
# is_gt.  Each chain is 3/5 ACT; the per-chain offset staggers routes so
# the engines see a mixed load at every instant (all chains on the same
# route bunches one engine and idles the other for ~650ns per step).
import os as _os
_RV = _os.environ.get("K_ROUTE", "coset")
if _RV == "coset":
    M2_ROUTE = [[(s + 2 * c) % 5 < 3 for c in range(4)] for s in range(5)]
elif _RV.startswith("c20q"):
    _q = int(_RV[4:])
    M2_ROUTE = [[(4 * s + c) % 20 < _q for c in range(4)]
                for s in range(20)]
else:
    M2_ROUTE = [
        [True, True, True, False],
        [True, False, False, True],
        [False, True, True, True],
        [True, False, True, False],
        [False, True, False, True],
    ]
PAIR_EMIT = _os.environ.get("K_PAIR", "1") == "1"
IO_BUFS = int(_os.environ.get("K_IOBUFS", "2"))
PRIME = int(_os.environ.get("K_PRIME", "5"))
VMM = _os.environ.get("K_VMM", "split2")  # split2 | bf16 | f32r
_SHTMP = int(_os.environ.get("K_SHTMP", "0"))
if _SHTMP:
    SHARED_TMP = True
    TMP_BUFS = _SHTMP



def _emit_tile_chain(nc, t, c, dram, wt, sb, ptmp, pz0, io, fstar, inv, cfac):
    """Generator emitting one packed tile's program; yields between steps
    so NCHAINS chains interleave in emission (and thus in the static
    per-engine schedules)."""
    _dyn = not isinstance(t, int)
    src = dram["inp0"][bass.ds(t, 1)][0, c] if _dyn else dram["inp0"][t][c]
    dst = dram["yout"][bass.ds(t, 1)][0, c] if _dyn else dram["yout"][t][c]
    # Host-side feature permutation puts f* at feature 0, so with the
    # interleaved layout (partition = 4*feature + group) the extraction
    # rows are partitions 0..3 (32-aligned base, required by the engines).
    assert fstar == 0
    inp = io.tile([2 * G, TILE_N], F32 if FWD_FP32 else F32R,
                  tag=f"inp{c}", name=f"inp_{c}")
    nc.sync.dma_start(out=inp[:], in_=src)

    z0 = pz0.tile([128, TILE_N], F32, tag="z0", name=f"z0_{c}")
    # z0 = blkdiag(W0) @ [x; y_mean]   (no bias; ACT adds b0 every step)
    nc.tensor.matmul(
        z0[:], wt["L0"][:], inp[:],
        start=True, stop=False, skip_group_check=True,
    )
    yield

    for s in range(STEPS):
        HDT = F32 if FWD_FP32 else F32R
        h0 = sb.tile([128, TILE_N], HDT, tag="h0", name=f"h0_{c}")
        nc.scalar.activation(h0[:], z0[:], AF.Relu, bias=wt["b0"][:])
        yield
        z1 = ptmp.tile([128, TILE_N], F32, tag="tmp", name=f"z1_{c}")
        nc.tensor.matmul(
            z1[:], wt["Lz1"][:], h0[:],
            start=True, stop=True, skip_group_check=True,
        )
        yield
        h1 = sb.tile([128, TILE_N], HDT, tag="h1", name=f"h1_{c}")
        nc.scalar.activation(h1[:], z1[:], AF.Relu, bias=wt["b1"][:])
        yield
        z2 = ptmp.tile([128, TILE_N], F32, tag="tmp", name=f"z2_{c}")
        nc.tensor.matmul(
            z2[:], wt["Lz2"][:], h1[:],
            start=True, stop=True, skip_group_check=True,
        )
        yield
        m2 = sb.tile([128, TILE_N], BF16, tag="m2", name=f"m2_{c}")
        # GpSimd tensor_scalar measured ~8us/op on HW - never use it.
        # The 0/1 mask is exact in bf16.  m2 = (z2 + b2 > 0) is computed in
        # ONE op either on ACT as sigmoid(2^100 * z2 + 2^100*b2) (sigmoid
        # saturates to exact 0.0/1.0 for |arg| > ~90, i.e. |z2+b2| >
        # ~7e-29) or on DVE as is_gt straight from PSUM; the split
        # balances ACT vs DVE load (~60/40).  The 5x4 table staggers the
        # routes across chains: every chain is 3/5 ACT, and the per-step
        # ACT-lane counts are [3,2,3,2,2] - putting all chains on the same
        # route bunches one engine and idles the other for ~650ns per step.
        if M2_ROUTE[s % len(M2_ROUTE)][c % 4]:
            nc.scalar.activation(m2[:], z2[:], AF.Sigmoid,
                                 bias=wt["sb2"][:], scale=MASK_SCALE)
        else:
            nc.vector.tensor_scalar(m2[:], z2[:], wt["nb2"][:], None,
                                    ALU.is_gt)
        yield
        # g1p = blkdiag(W2 * W3^T) @ m2 in split-bf16 (hi + lo residual,
        # ~16-bit effective weights - better than fp32r).
        g1p = ptmp.tile([128, TILE_N], F32, tag="tmp", name=f"g1p_{c}")
        if VMM == "split2":
            nc.tensor.matmul(
                g1p[:], wt["Lg1h"][:], m2[:],
                start=True, stop=False, skip_group_check=True,
            )
            nc.tensor.matmul(
                g1p[:], wt["Lg1l"][:], m2[:],
                start=False, stop=True, skip_group_check=True,
            )
        else:
            nc.tensor.matmul(
                g1p[:], wt["Lg1h" if VMM == "bf16" else "Lg1f"][:], m2[:],
                start=True, stop=True, skip_group_check=True,
            )
        yield
        g1 = sb.tile([128, TILE_N], F32R, tag="g1", name=f"g1_{c}")
        nc.vector.scalar_tensor_tensor(
            g1[:], h1[:], 0.0, g1p[:], op0=ALU.is_gt, op1=ALU.mult
        )
        yield
        g0p = ptmp.tile([128, TILE_N], F32, tag="tmp", name=f"g0p_{c}")
        nc.tensor.matmul(
            g0p[:], wt["Lg0"][:], g1[:],
            start=True, stop=True, skip_group_check=True,
        )
        yield
        g0 = sb.tile([128, TILE_N], F32R, tag="g0", name=f"g0_{c}")
        nc.vector.scalar_tensor_tensor(
            g0[:], h0[:], 0.0, g0p[:], op0=ALU.is_gt, op1=ALU.mult
        )
        yield
        # z0 += blkdiag(P) @ g0  == w0y (x) dy for this step
        nc.tensor.matmul(
            z0[:], wt["LP"][:], g0[:],
            start=False, stop=False, skip_group_check=True,
        )
        yield

    # Final extraction: y = inv * (z0_fin[f*] - W0[f*,0]*x)  (y_mean is
    # inside z0 via the y-column of the init matmul).  The x-subtraction
    # is folded into the z0 bank by one extra matmul on the inp tile, so
    # the whole extraction is a single ACT copy-with-scale from PSUM -
    # no xt fetch, no DVE op at the tile boundary.
    nc.tensor.matmul(
        z0[0:G, :], wt["Lyx"][:], inp[:],
        start=False, stop=True, skip_group_check=True,
    )
    yield
    yo = io.tile([G, TILE_N], F32, tag=f"yo{c}", name=f"yo_{c}")
    # yo = z0[f*-rows] * inv   (also releases the z0 bank)
    nc.scalar.activation(yo[:], z0[0:G, :], AF.Copy, scale=inv)
    yield
    nc.sync.dma_start(out=dst, in_=yo[:])
    yield


def build(nt=NT_FULL, fstar=0, inv=1.0, cfac=1.0, reps=1, dyn=None):
    """Build + compile the per-core Bass program for nt packed tiles."""
    nc = bacc.Bacc("TRN2", target_bir_lowering=False, debug=False,
                   num_devices=NCORES)

    ntq = nt // NCHAINS
    dram = {
        "inp0": nc.dram_tensor("inp0", [ntq, NCHAINS, 2 * G, TILE_N],
                               F32 if FWD_FP32 else F32R,
                               kind="ExternalInput").ap(),
        "yout": nc.dram_tensor("yout", [ntq, NCHAINS, G, TILE_N], F32,
                               kind="ExternalOutput").ap(),
    }
    wspec = {
        "L0": [2 * G, 128],
        "Lyx": [2 * G, G],
        "Lz1": [128, 128], "Lz2": [128, 128],
        "Lg1h": [128, 128], "Lg1l": [128, 128], "Lg1f": [128, 128],
        "Lg0": [128, 128],
        "LP": [128, 128],
        "b0": [128, 1], "b1": [128, 1], "b2": [128, 1], "nb2": [128, 1],
        "sb2": [128, 1],
        "ym": [G, 1],
    }
    fwd = F32 if FWD_FP32 else F32R
    wdtype = {k: (F32 if k in ("b0", "b1", "b2", "nb2", "sb2", "ym") else
                  (BF16 if k in ("Lg1h", "Lg1l") else
                   (fwd if k in ("Lz1", "Lz2", "L0", "Lyx") else F32R)))
              for k in wspec}
    wdram = {k: nc.dram_tensor(f"w_{k}", sh, wdtype[k],
                               kind="ExternalInput").ap()
             for k, sh in wspec.items()}

    with tile.TileContext(nc) as tc:
        import contextlib
        with contextlib.ExitStack() as ctx:
            wp = ctx.enter_context(tc.tile_pool(name="wp", bufs=1))
            io = ctx.enter_context(tc.tile_pool(name="io", bufs=IO_BUFS))
            sbs = [ctx.enter_context(tc.tile_pool(name=f"sb{c}", bufs=2))
                   for c in range(NCHAINS)]
            if SHARED_TMP:
                pt = ctx.enter_context(
                    tc.tile_pool(name="pt", bufs=TMP_BUFS, space="PSUM"))
                ptmps = [pt] * NCHAINS
            else:
                ptmps = [ctx.enter_context(
                    tc.tile_pool(name=f"pt{c}", bufs=1, space="PSUM"))
                    for c in range(NCHAINS)]
            pz0s = [ctx.enter_context(
                tc.tile_pool(name=f"pz{c}", bufs=1, space="PSUM"))
                for c in range(NCHAINS)]

            wt = {}
            for k, sh in wspec.items():
                wt[k] = wp.tile(sh, wdtype[k], tag=f"w_{k}", name=f"wt_{k}")
                nc.sync.dma_start(out=wt[k][:], in_=wdram[k][:])

            assert nt % NCHAINS == 0

            def emit_pair(t0, t1):
                # Each lane runs its tile in quad t0 then its tile in quad
                # t1 as ONE continuous generator, so a lane's tile boundary
                # (extraction + restart) is surrounded in every engine's
                # program order by the OTHER lanes' steady-state step ops -
                # bunching all 4 boundaries together starves ACT/DVE for
                # ~650ns per boundary.
                def lane(c):
                    yield from _emit_tile_chain(nc, t0, c, dram, wt,
                                               sbs[c], ptmps[c], pz0s[c],
                                               io, fstar, inv, cfac)
                    yield from _emit_tile_chain(nc, t1, c, dram, wt,
                                               sbs[c], ptmps[c], pz0s[c],
                                               io, fstar, inv, cfac)

                def quad_lane(c, t):
                    yield from _emit_tile_chain(nc, t, c, dram, wt,
                                               sbs[c], ptmps[c], pz0s[c],
                                               io, fstar, inv, cfac)
                if PAIR_EMIT:
                    chains = [lane(c) for c in range(NCHAINS)]
                else:
                    chains = None
                # phase-offset the chains by ~1/NCHAINS of a step so no
                # engine sees two dependent ops of one chain back-to-back
                prime = PRIME
                if chains is None:
                    for t in (t0, t1):
                        chs = [quad_lane(c, t) for c in range(NCHAINS)]
                        for c, ch in enumerate(chs):
                            for _ in range(c * prime):
                                next(ch)
                        alive = list(chs)
                        while alive:
                            for ch in list(alive):
                                try:
                                    next(ch)
                                except StopIteration:
                                    alive.remove(ch)
                    return
                for c, ch in enumerate(chains):
                    for _ in range(c * prime):
                        next(ch)
                alive = list(chains)
                while alive:
                    for ch in list(alive):
                        try:
                            next(ch)
                        except StopIteration:
                            alive.remove(ch)

            use_dyn = DYN if dyn is None else dyn
            if use_dyn:
                def body():
                    assert ntq % 2 == 0
                    with tc.For_i(0, ntq // 2, 1,
                                  hint_engines=(mybir.EngineType.PE,)) as iv:
                        iv2 = iv * 2
                        emit_pair(iv2, iv2 + 1)
                if reps > 1:
                    with tc.For_i(0, reps, 1):
                        body()
                else:
                    body()
            else:
                for tq in range(0, ntq, 2):
                    emit_pair(tq, tq + 1)

    nc.compile()
    return nc


def make_weight_arrays(W0, b0, W1, b1, W2, b2, W3, b3, y_mean):
    """Host-side constant construction (all small)."""
    import ml_dtypes
    LR0 = MASK_SCALE
    # Layer-0 features are permuted so f* = argmax |W0[:,1]| is feature 0
    # (the extraction rows land on 32-aligned partitions 0..3), then laid
    # out interleaved: partition = 4*feature + group, so blkdiag weights
    # are kron(A, I_G) and per-feature biases repeat 4x.
    W0 = np.asarray(W0, np.float32)
    f_raw = int(np.argmax(np.abs(W0[:, 1])))
    perm = [f_raw] + [j for j in range(W) if j != f_raw]
    W0 = W0[perm]
    b0 = np.asarray(b0, np.float32)[perm]
    W1 = np.asarray(W1, np.float32)[:, perm]
    eye = np.eye(G, dtype=np.float32)
    blk = lambda A: np.kron(A.astype(np.float32), eye)
    rep = lambda v: np.repeat(v.astype(np.float32), G)[:, None]
    w0y = W0[:, 1].astype(np.float32)
    P = (-LR) * np.outer(w0y, w0y)
    ym = np.float32(np.asarray(y_mean).reshape(-1)[0])
    V = blk(W2 * W3[0][:, None])
    Vh = V.astype(ml_dtypes.bfloat16)
    Vl = (V - Vh.astype(np.float32)).astype(ml_dtypes.bfloat16)
    L0 = np.zeros((2 * G, 128), np.float32)
    for g in range(G):
        L0[2 * g, g::G] = W0[:, 0]          # x_g feeds z0[4j+g]
        L0[2 * g + 1, g::G] = W0[:, 1]      # y_g feeds z0[4j+g]
    # End-of-tile extraction helper: z0[f*-rows] -= W0[f*,0] * x, leaving
    # w0y[f*] * y in the f*-rows (f* is feature 0 after the permutation).
    Lyx = np.zeros((2 * G, G), np.float32)
    for g in range(G):
        Lyx[2 * g, g] = -W0[0, 0]
    out = {
        "w_L0": L0,                              # [8, 128]
        "w_Lyx": Lyx,                            # [8, 4]
        "w_Lz1": blk(W1.T),                      # [128, 128]
        "w_Lz2": blk(W2.T),                      # [128, 128]
        "w_Lg1h": None, "w_Lg1l": None,          # filled below (bf16 pair)
        "w_Lg1f": V,                             # fp32r single-MM variant
        "w_Lg0": blk(W1),                        # [128, 128]
        "w_LP": blk(P),                          # [128, 128]
        "w_b0": rep(b0),
        "w_b1": rep(b1),
        "w_b2": rep(b2),
        "w_nb2": rep(-b2),
        "w_sb2": np.clip(np.repeat(b2.astype(np.float64), G)[:, None] * LR0,
                         -3e38, 3e38).astype(np.float32),
        "w_ym": np.full((G, 1), ym, np.float32),
    }
    out["w_Lg1h"] = Vh
    out["w_Lg1l"] = Vl
    return out


def extraction_consts(W0):
    """inv/cfac for the f* feature; fstar returned as 0 because
    make_weight_arrays permutes f* to feature position 0."""
    W0 = np.asarray(W0, np.float32)
    f_raw = int(np.argmax(np.abs(W0[:, 1])))
    inv = float(1.0 / W0[f_raw, 1])
    cfac = float(W0[f_raw, 0] * inv)
    return 0, inv, cfac


def make_core_inputs(x, y_mean, nt=NT_FULL):
    """Per-core input tiles: [nt, 8, 512] with x on even rows, y_mean on
    odd rows.  Returns a list of NCORES arrays."""
    xs = np.ascontiguousarray(
        np.asarray(x, np.float32).reshape(NCORES, nt, G, TILE_N))
    ym = np.float32(np.asarray(y_mean).reshape(-1)[0])
    maps = []
    for c in range(NCORES):
        inp0 = np.empty((nt, 2 * G, TILE_N), dtype=np.float32)
        inp0[:, 0::2, :] = xs[c]
        inp0[:, 1::2, :] = ym
        maps.append(inp0.reshape(nt // NCHAINS, NCHAINS, 2 * G, TILE_N))
    return maps


_NC_CACHE = {}


def get_nc(nt, fstar, inv, cfac):
    key = (nt, fstar, round(inv, 9), round(cfac, 9))
    if key not in _NC_CACHE:
        _NC_CACHE[key] = build(nt, fstar, inv, cfac)
    return _NC_CACHE[key]


def kernel(x, W0, b0, W1, b1, W2, b2, W3, b3, y_mean):
    x = np.asarray(x, dtype=np.float32)
    fstar, inv, cfac = extraction_consts(W0)
    nc = get_nc(NT_FULL, fstar, inv, cfac)

    warr = make_weight_arrays(
        np.asarray(W0), np.asarray(b0), np.asarray(W1), np.asarray(b1),
        np.asarray(W2), np.asarray(b2), np.asarray(W3), np.asarray(b3),
        np.asarray(y_mean))
    inp0s = make_core_inputs(x, np.asarray(y_mean), NT_FULL)
    in_maps = [{"inp0": inp0s[c], **warr} for c in range(NCORES)]

    res = run_bass_kernel_spmd(nc, in_maps, list(range(NCORES)))
    youts = [res.results[c]["yout"].reshape(BC) for c in range(NCORES)]
    return np.concatenate(youts).reshape(B, 1).astype(np.float32)



# revision 48
# speedup vs baseline: 1.0192x; 1.0037x over previous
"""Trainium2 Bass kernel for nn_BaseEBM (EBM inner gradient-descent loop).

Computation (per sample, matching the reference):
    y = y_mean
    repeat 20x:  y <- y - 0.1 * dE/dy
    E(x, y) = W3 @ relu(W2 @ relu(W1 @ relu(W0 @ [x, y] + b0) + b1) + b2) + b3

Distribution: pure data parallel over 8 NeuronCores (131072 samples each).

Device layout: feature-major [128, 512] tiles. Four independent sample
groups of 32 features are packed INTERLEAVED across the 128 partitions
(partition = 4*feature + group); 512 samples per group per tile -> 2048
samples/tile, 64 tiles/core. All matmuls use kron(A, I4) weights so one
instruction processes 4 groups at full PE rate (fp32r, 1 cycle/column).

Key algebraic restructurings:
  * The energy head (layer 3 forward) is never computed; W3 is folded into
    the first backward matmul: g1p = blkdiag(W2 * W3^T) @ m2.
  * x never changes across steps, so z0 = W0 @ [x, y] is kept resident in
    PSUM for all 20 steps and updated by accumulating matmuls:
        dz0 = -lr * w0y (w0y . g0) = blkdiag(P) @ g0,  P = -lr w0y w0y^T.
  * y is never materialized during the loop.  Since dz0 = w0y * dy, the
    final y is recovered from the PSUM residue:
        y = (z0_fin[f*] - W0[f*,0] x) / w0y[f*]   (y_mean is inside z0).
    This frees a PSUM bank per chain, allowing FOUR independent
    tile-chains in the 8 banks - needed because one chain's per-step
    dependency path (~4.5 us) is ~3x its per-engine work (~1.6 us).
  * Layer-0 features are host-permuted so f* is feature 0; with the
    interleaved layout the f*-rows are partitions 0..3, so the final y is
    one DVE op reading the z0 bank directly (no 128-row copy / gathers).
  * Masked backprop uses the fused DVE op (h > 0) * g in one instruction
    (scalar_tensor_tensor with is_gt + mult), so relu masks are never
    materialized for layers 0/1.
  * The layer-2 mask is ONE op either way: ACT sigmoid(2^100 * z2) (exact
    0/1 - sigmoid saturates) or DVE is_gt from PSUM; a per-chain-offset
    60/40 route split balances ACT vs DVE load at every instant.
  * The transient-PSUM pool is bufs=1 per chain: the slot-reuse deps
    exactly coincide with the data deps (z2 can only start after h1,
    which is when z1's bank frees), so one bank per chain costs nothing.

Per step per tile: 6 matmuls (V in split-bf16: hi+lo, needed for the
2e-2 error budget), 2 ACT relus, 1 one-op layer-2 mask, 2 fused DVE
mask-multiplies.  ACT and DVE are the bottleneck engines (~87% busy in
the cost-model timeline); PE ~67%.
"""

import numpy as np

import concourse.bass as bass
import concourse.mybir as mybir
import concourse.tile as tile
from concourse import bacc
from concourse.bass_utils import run_bass_kernel_spmd

F32 = mybir.dt.float32
F32R = mybir.dt.float32r
BF16 = mybir.dt.bfloat16
ALU = mybir.AluOpType
AF = mybir.ActivationFunctionType

B = 1048576
NCORES = 8
BC = B // NCORES           # 131072 samples per core
G = 4                      # sample groups packed across partitions
TILE_N = 512               # samples per group per tile (PSUM bank limit)
SPT = G * TILE_N           # 2048 samples per tile
NT_FULL = BC // SPT        # 64 tiles per core
STEPS = 20
LR = 0.1
W = 32
NCHAINS = 4
SHARED_TMP = False
TMP_BUFS = 3
DYN = True      # hardware For_i loop over tile-quads
FWD_FP32 = False  # true-fp32 forward matmuls (4 cyc/row) -> ~40x less error
MASK_SCALE = float(2.0 ** 100)  # sigmoid(MASK_SCALE*z) == (z > 0) exactly
# m2-route table [step % 5][chain % 4]: True = ACT sigmoid, False = DVE
# is_gt.  Each chain is 3/5 ACT; the per-chain offset staggers routes so
# the engines see a mixed load at every instant (all chains on the same
# route bunches one engine and idles the other for ~650ns per step).
import os as _os
_RV = _os.environ.get("K_ROUTE", "coset")
if _RV == "coset":
    M2_ROUTE = [[(s + 2 * c) % 5 < 3 for c in range(4)] for s in range(5)]
elif _RV.startswith("c20q"):
    _q = int(_RV[4:])
    M2_ROUTE = [[(4 * s + c) % 20 < _q for c in range(4)]
                for s in range(20)]
else:
    M2_ROUTE = [
        [True, True, True, False],
        [True, False, False, True],
        [False, True, True, True],
        [True, False, True, False],
        [False, True, False, True],
    ]
PAIR_EMIT = _os.environ.get("K_PAIR", "1") == "1"
IO_BUFS = int(_os.environ.get("K_IOBUFS", "2"))
PRIME = int(_os.environ.get("K_PRIME", "5"))
VMM = _os.environ.get("K_VMM", "split2")  # split2 | bf16 | f32r
_SHTMP = int(_os.environ.get("K_SHTMP", "0"))
if _SHTMP:
    SHARED_TMP = True
    TMP_BUFS = _SHTMP



def _fetch_inp(nc, t, c, dram, io):
    """Acquire + fetch a tile's input; emitted early so the DMA latency
    hides behind the previous tile's steps instead of sitting on the
    lane-restart path."""
    _dyn = not isinstance(t, int)
    src = dram["inp0"][bass.ds(t, 1)][0, c] if _dyn else dram["inp0"][t][c]
    inp = io.tile([2 * G, TILE_N], F32 if FWD_FP32 else F32R,
                  tag=f"inp{c}", name=f"inp_{c}")
    nc.sync.dma_start(out=inp[:], in_=src)
    return inp


def _emit_tile_chain(nc, t, c, dram, wt, sb, ptmp, pz0, io, fstar, inv,
                     cfac, inp_pre=None):
    """Generator emitting one packed tile's program; yields between steps
    so NCHAINS chains interleave in emission (and thus in the static
    per-engine schedules)."""
    _dyn = not isinstance(t, int)
    dst = dram["yout"][bass.ds(t, 1)][0, c] if _dyn else dram["yout"][t][c]
    # Host-side feature permutation puts f* at feature 0, so with the
    # interleaved layout (partition = 4*feature + group) the extraction
    # rows are partitions 0..3 (32-aligned base, required by the engines).
    assert fstar == 0
    inp = inp_pre if inp_pre is not None else _fetch_inp(nc, t, c, dram, io)

    z0 = pz0.tile([128, TILE_N], F32, tag="z0", name=f"z0_{c}")
    # z0 = blkdiag(W0) @ [x; y_mean]   (no bias; ACT adds b0 every step)
    nc.tensor.matmul(
        z0[:], wt["L0"][:], inp[:],
        start=True, stop=False, skip_group_check=True,
    )
    yield

    for s in range(STEPS):
        HDT = F32 if FWD_FP32 else F32R
        h0 = sb.tile([128, TILE_N], HDT, tag="h0", name=f"h0_{c}")
        nc.scalar.activation(h0[:], z0[:], AF.Relu, bias=wt["b0"][:])
        yield
        z1 = ptmp.tile([128, TILE_N], F32, tag="tmp", name=f"z1_{c}")
        nc.tensor.matmul(
            z1[:], wt["Lz1"][:], h0[:],
            start=True, stop=True, skip_group_check=True,
        )
        yield
        h1 = sb.tile([128, TILE_N], HDT, tag="h1", name=f"h1_{c}")
        nc.scalar.activation(h1[:], z1[:], AF.Relu, bias=wt["b1"][:])
        yield
        z2 = ptmp.tile([128, TILE_N], F32, tag="tmp", name=f"z2_{c}")
        nc.tensor.matmul(
            z2[:], wt["Lz2"][:], h1[:],
            start=True, stop=True, skip_group_check=True,
        )
        yield
        m2 = sb.tile([128, TILE_N], BF16, tag="m2", name=f"m2_{c}")
        # GpSimd tensor_scalar measured ~8us/op on HW - never use it.
        # The 0/1 mask is exact in bf16.  m2 = (z2 + b2 > 0) is computed in
        # ONE op either on ACT as sigmoid(2^100 * z2 + 2^100*b2) (sigmoid
        # saturates to exact 0.0/1.0 for |arg| > ~90, i.e. |z2+b2| >
        # ~7e-29) or on DVE as is_gt straight from PSUM; the split
        # balances ACT vs DVE load (~60/40).  The 5x4 table staggers the
        # routes across chains: every chain is 3/5 ACT, and the per-step
        # ACT-lane counts are [3,2,3,2,2] - putting all chains on the same
        # route bunches one engine and idles the other for ~650ns per step.
        if M2_ROUTE[s % len(M2_ROUTE)][c % 4]:
            nc.scalar.activation(m2[:], z2[:], AF.Sigmoid,
                                 bias=wt["sb2"][:], scale=MASK_SCALE)
        else:
            nc.vector.tensor_scalar(m2[:], z2[:], wt["nb2"][:], None,
                                    ALU.is_gt)
        yield
        # g1p = blkdiag(W2 * W3^T) @ m2 in split-bf16 (hi + lo residual,
        # ~16-bit effective weights - better than fp32r).
        g1p = ptmp.tile([128, TILE_N], F32, tag="tmp", name=f"g1p_{c}")
        if VMM == "split2":
            nc.tensor.matmul(
                g1p[:], wt["Lg1h"][:], m2[:],
                start=True, stop=False, skip_group_check=True,
            )
            nc.tensor.matmul(
                g1p[:], wt["Lg1l"][:], m2[:],
                start=False, stop=True, skip_group_check=True,
            )
        else:
            nc.tensor.matmul(
                g1p[:], wt["Lg1h" if VMM == "bf16" else "Lg1f"][:], m2[:],
                start=True, stop=True, skip_group_check=True,
            )
        yield
        g1 = sb.tile([128, TILE_N], F32R, tag="g1", name=f"g1_{c}")
        nc.vector.scalar_tensor_tensor(
            g1[:], h1[:], 0.0, g1p[:], op0=ALU.is_gt, op1=ALU.mult
        )
        yield
        g0p = ptmp.tile([128, TILE_N], F32, tag="tmp", name=f"g0p_{c}")
        nc.tensor.matmul(
            g0p[:], wt["Lg0"][:], g1[:],
            start=True, stop=True, skip_group_check=True,
        )
        yield
        g0 = sb.tile([128, TILE_N], F32R, tag="g0", name=f"g0_{c}")
        nc.vector.scalar_tensor_tensor(
            g0[:], h0[:], 0.0, g0p[:], op0=ALU.is_gt, op1=ALU.mult
        )
        yield
        # z0 += blkdiag(P) @ g0  == w0y (x) dy for this step
        nc.tensor.matmul(
            z0[:], wt["LP"][:], g0[:],
            start=False, stop=False, skip_group_check=True,
        )
        yield

    # Final extraction: y = inv * (z0_fin[f*] - W0[f*,0]*x)  (y_mean is
    # inside z0 via the y-column of the init matmul).  The x-subtraction
    # is folded into the z0 bank by one extra matmul on the inp tile, so
    # the whole extraction is a single ACT copy-with-scale from PSUM -
    # no xt fetch, no DVE op at the tile boundary.
    nc.tensor.matmul(
        z0[0:G, :], wt["Lyx"][:], inp[:],
        start=False, stop=True, skip_group_check=True,
    )
    yield
    yo = io.tile([G, TILE_N], F32, tag=f"yo{c}", name=f"yo_{c}")
    # yo = z0[f*-rows] * inv   (also releases the z0 bank)
    nc.scalar.activation(yo[:], z0[0:G, :], AF.Copy, scale=inv)
    yield
    nc.sync.dma_start(out=dst, in_=yo[:])
    yield


def build(nt=NT_FULL, fstar=0, inv=1.0, cfac=1.0, reps=1, dyn=None):
    """Build + compile the per-core Bass program for nt packed tiles."""
    nc = bacc.Bacc("TRN2", target_bir_lowering=False, debug=False,
                   num_devices=NCORES)

    ntq = nt // NCHAINS
    dram = {
        "inp0": nc.dram_tensor("inp0", [ntq, NCHAINS, 2 * G, TILE_N],
                               F32 if FWD_FP32 else F32R,
                               kind="ExternalInput").ap(),
        "yout": nc.dram_tensor("yout", [ntq, NCHAINS, G, TILE_N], F32,
                               kind="ExternalOutput").ap(),
    }
    wspec = {
        "L0": [2 * G, 128],
        "Lyx": [2 * G, G],
        "Lz1": [128, 128], "Lz2": [128, 128],
        "Lg1h": [128, 128], "Lg1l": [128, 128], "Lg1f": [128, 128],
        "Lg0": [128, 128],
        "LP": [128, 128],
        "b0": [128, 1], "b1": [128, 1], "b2": [128, 1], "nb2": [128, 1],
        "sb2": [128, 1],
        "ym": [G, 1],
    }
    fwd = F32 if FWD_FP32 else F32R
    wdtype = {k: (F32 if k in ("b0", "b1", "b2", "nb2", "sb2", "ym") else
                  (BF16 if k in ("Lg1h", "Lg1l") else
                   (fwd if k in ("Lz1", "Lz2", "L0", "Lyx") else F32R)))
              for k in wspec}
    wdram = {k: nc.dram_tensor(f"w_{k}", sh, wdtype[k],
                               kind="ExternalInput").ap()
             for k, sh in wspec.items()}

    with tile.TileContext(nc) as tc:
        import contextlib
        with contextlib.ExitStack() as ctx:
            wp = ctx.enter_context(tc.tile_pool(name="wp", bufs=1))
            io = ctx.enter_context(tc.tile_pool(name="io", bufs=IO_BUFS))
            sbs = [ctx.enter_context(tc.tile_pool(name=f"sb{c}", bufs=2))
                   for c in range(NCHAINS)]
            if SHARED_TMP:
                pt = ctx.enter_context(
                    tc.tile_pool(name="pt", bufs=TMP_BUFS, space="PSUM"))
                ptmps = [pt] * NCHAINS
            else:
                ptmps = [ctx.enter_context(
                    tc.tile_pool(name=f"pt{c}", bufs=1, space="PSUM"))
                    for c in range(NCHAINS)]
            pz0s = [ctx.enter_context(
                tc.tile_pool(name=f"pz{c}", bufs=1, space="PSUM"))
                for c in range(NCHAINS)]

            wt = {}
            for k, sh in wspec.items():
                wt[k] = wp.tile(sh, wdtype[k], tag=f"w_{k}", name=f"wt_{k}")
                nc.sync.dma_start(out=wt[k][:], in_=wdram[k][:])

            assert nt % NCHAINS == 0

            def emit_pair(t0, t1):
                # Each lane runs its tile in quad t0 then its tile in quad
                # t1 as ONE continuous generator, so a lane's tile boundary
                # (extraction + restart) is surrounded in every engine's
                # program order by the OTHER lanes' steady-state step ops -
                # bunching all 4 boundaries together starves ACT/DVE for
                # ~650ns per boundary.
                def lane(c):
                    inp0_t = _fetch_inp(nc, t0, c, dram, io)
                    inp1_t = _fetch_inp(nc, t1, c, dram, io)
                    yield from _emit_tile_chain(nc, t0, c, dram, wt,
                                               sbs[c], ptmps[c], pz0s[c],
                                               io, fstar, inv, cfac,
                                               inp_pre=inp0_t)
                    yield from _emit_tile_chain(nc, t1, c, dram, wt,
                                               sbs[c], ptmps[c], pz0s[c],
                                               io, fstar, inv, cfac,
                                               inp_pre=inp1_t)

                def quad_lane(c, t):
                    yield from _emit_tile_chain(nc, t, c, dram, wt,
                                               sbs[c], ptmps[c], pz0s[c],
                                               io, fstar, inv, cfac)
                if PAIR_EMIT:
                    chains = [lane(c) for c in range(NCHAINS)]
                else:
                    chains = None
                # phase-offset the chains by ~1/NCHAINS of a step so no
                # engine sees two dependent ops of one chain back-to-back
                prime = PRIME
                if chains is None:
                    for t in (t0, t1):
                        chs = [quad_lane(c, t) for c in range(NCHAINS)]
                        for c, ch in enumerate(chs):
                            for _ in range(c * prime):
                                next(ch)
                        alive = list(chs)
                        while alive:
                            for ch in list(alive):
                                try:
                                    next(ch)
                                except StopIteration:
                                    alive.remove(ch)
                    return
                for c, ch in enumerate(chains):
                    for _ in range(c * prime):
                        next(ch)
                alive = list(chains)
                while alive:
                    for ch in list(alive):
                        try:
                            next(ch)
                        except StopIteration:
                            alive.remove(ch)

            use_dyn = DYN if dyn is None else dyn
            if use_dyn:
                def body():
                    assert ntq % 2 == 0
                    with tc.For_i(0, ntq // 2, 1,
                                  hint_engines=(mybir.EngineType.PE,)) as iv:
                        iv2 = iv * 2
                        emit_pair(iv2, iv2 + 1)
                if reps > 1:
                    with tc.For_i(0, reps, 1):
                        body()
                else:
                    body()
            else:
                for tq in range(0, ntq, 2):
                    emit_pair(tq, tq + 1)

    nc.compile()
    return nc


def make_weight_arrays(W0, b0, W1, b1, W2, b2, W3, b3, y_mean):
    """Host-side constant construction (all small)."""
    import ml_dtypes
    LR0 = MASK_SCALE
    # Layer-0 features are permuted so f* = argmax |W0[:,1]| is feature 0
    # (the extraction rows land on 32-aligned partitions 0..3), then laid
    # out interleaved: partition = 4*feature + group, so blkdiag weights
    # are kron(A, I_G) and per-feature biases repeat 4x.
    W0 = np.asarray(W0, np.float32)
    f_raw = int(np.argmax(np.abs(W0[:, 1])))
    perm = [f_raw] + [j for j in range(W) if j != f_raw]
    W0 = W0[perm]
    b0 = np.asarray(b0, np.float32)[perm]
    W1 = np.asarray(W1, np.float32)[:, perm]
    eye = np.eye(G, dtype=np.float32)
    blk = lambda A: np.kron(A.astype(np.float32), eye)
    rep = lambda v: np.repeat(v.astype(np.float32), G)[:, None]
    w0y = W0[:, 1].astype(np.float32)
    P = (-LR) * np.outer(w0y, w0y)
    ym = np.float32(np.asarray(y_mean).reshape(-1)[0])
    V = blk(W2 * W3[0][:, None])
    Vh = V.astype(ml_dtypes.bfloat16)
    Vl = (V - Vh.astype(np.float32)).astype(ml_dtypes.bfloat16)
    L0 = np.zeros((2 * G, 128), np.float32)
    for g in range(G):
        L0[2 * g, g::G] = W0[:, 0]          # x_g feeds z0[4j+g]
        L0[2 * g + 1, g::G] = W0[:, 1]      # y_g feeds z0[4j+g]
    # End-of-tile extraction helper: z0[f*-rows] -= W0[f*,0] * x, leaving
    # w0y[f*] * y in the f*-rows (f* is feature 0 after the permutation).
    Lyx = np.zeros((2 * G, G), np.float32)
    for g in range(G):
        Lyx[2 * g, g] = -W0[0, 0]
    out = {
        "w_L0": L0,                              # [8, 128]
        "w_Lyx": Lyx,                            # [8, 4]
        "w_Lz1": blk(W1.T),                      # [128, 128]
        "w_Lz2": blk(W2.T),                      # [128, 128]
        "w_Lg1h": None, "w_Lg1l": None,          # filled below (bf16 pair)
        "w_Lg1f": V,                             # fp32r single-MM variant
        "w_Lg0": blk(W1),                        # [128, 128]
        "w_LP": blk(P),                          # [128, 128]
        "w_b0": rep(b0),
        "w_b1": rep(b1),
        "w_b2": rep(b2),
        "w_nb2": rep(-b2),
        "w_sb2": np.clip(np.repeat(b2.astype(np.float64), G)[:, None] * LR0,
                         -3e38, 3e38).astype(np.float32),
        "w_ym": np.full((G, 1), ym, np.float32),
    }
    out["w_Lg1h"] = Vh
    out["w_Lg1l"] = Vl
    return out


def extraction_consts(W0):
    """inv/cfac for the f* feature; fstar returned as 0 because
    make_weight_arrays permutes f* to feature position 0."""
    W0 = np.asarray(W0, np.float32)
    f_raw = int(np.argmax(np.abs(W0[:, 1])))
    inv = float(1.0 / W0[f_raw, 1])
    cfac = float(W0[f_raw, 0] * inv)
    return 0, inv, cfac


def make_core_inputs(x, y_mean, nt=NT_FULL):
    """Per-core input tiles: [nt, 8, 512] with x on even rows, y_mean on
    odd rows.  Returns a list of NCORES arrays."""
    xs = np.ascontiguousarray(
        np.asarray(x, np.float32).reshape(NCORES, nt, G, TILE_N))
    ym = np.float32(np.asarray(y_mean).reshape(-1)[0])
    maps = []
    for c in range(NCORES):
        inp0 = np.empty((nt, 2 * G, TILE_N), dtype=np.float32)
        inp0[:, 0::2, :] = xs[c]
        inp0[:, 1::2, :] = ym
        maps.append(inp0.reshape(nt // NCHAINS, NCHAINS, 2 * G, TILE_N))
    return maps


_NC_CACHE = {}


def get_nc(nt, fstar, inv, cfac):
    key = (nt, fstar, round(inv, 9), round(cfac, 9))
    if key not in _NC_CACHE:
        _NC_CACHE[key] = build(nt, fstar, inv, cfac)
    return _NC_CACHE[key]


def kernel(x, W0, b0, W1, b1, W2, b2, W3, b3, y_mean):
    x = np.asarray(x, dtype=np.float32)
    fstar, inv, cfac = extraction_consts(W0)
    nc = get_nc(NT_FULL, fstar, inv, cfac)

    warr = make_weight_arrays(
        np.asarray(W0), np.asarray(b0), np.asarray(W1), np.asarray(b1),
        np.asarray(W2), np.asarray(b2), np.asarray(W3), np.asarray(b3),
        np.asarray(y_mean))
    inp0s = make_core_inputs(x, np.asarray(y_mean), NT_FULL)
    in_maps = [{"inp0": inp0s[c], **warr} for c in range(NCORES)]

    res = run_bass_kernel_spmd(nc, in_maps, list(range(NCORES)))
    youts = [res.results[c]["yout"].reshape(BC) for c in range(NCORES)]
    return np.concatenate(youts).reshape(B, 1).astype(np.float32)

